# revision 11
# baseline (speedup 1.0000x reference)
"""GPT-2-style forward pass on 8 Trainium2 NeuronCores.

Sharding: DP=2 over batch x SP=4 over token chunks (interleaved {r, 7-r}
per rank for causal load balance). One AllGather of (kT, v) per layer
within each 4-core group; vocab-sharded lm_head after a final AllGather.
All matmuls in float32r; residual stream fp32. LN gains/biases are folded
into the adjacent matmul weights on the host (exact). The SPMD program is
rank-uniform: causal structure is expressed via per-rank 0/1 mask data.
"""
import sys

sys.path.insert(0, "/opt/trn_rl_repo")

import numpy as np
import concourse.bass as bass
import concourse.mybir as mybir
import concourse.bacc as bacc
import concourse.tile as tile
from concourse.bass_utils import run_bass_kernel_spmd
from concourse.masks import make_identity

f32 = mybir.dt.float32
f32r = mybir.dt.float32r
AF = mybir.ActivationFunctionType
ALU = mybir.AluOpType

V, S, C, H, L = 50257, 1024, 768, 12, 6
FF, HD = 3072, 64
B, T = 2, 1024
NCH = 8                   # token chunks per batch element
NCC = C // 128            # 6 contraction chunks over C
NFB = FF // 128           # 24 FFN blocks
NW = 512                  # lm_head vocab tile width
NN = 25                   # lm_head vocab tiles per rank
VSH = NW * NN             # 12800 vocab shard per rank
VPAD = 4 * VSH            # 51200
SCALE = float(C) ** -0.5
EPS = 1e-5
TRUE = [0, 7, 1, 6, 2, 5, 3, 4]  # gather slot -> true chunk id
KTE = C * 256             # elems of one rank's kT (768x256)
VE = 256 * C              # elems of one rank's v (256x768)
KVE = KTE + VE

_NLAYER = L               # dev override hook
_cache = {}


def _build(nlayer, use_bias):
    nc = bacc.Bacc("TRN2", target_bir_lowering=False, debug=False,
                   enable_asserts=False, num_devices=8)
    NL = nlayer

    d_x0 = nc.dram_tensor("x0", [2, 128, C], f32, kind="ExternalInput")
    d_wq = nc.dram_tensor("wq", [NL, NCC, 128, C], f32r, kind="ExternalInput")
    d_wk = nc.dram_tensor("wk", [NL, NCC, 128, C], f32r, kind="ExternalInput")
    d_wv = nc.dram_tensor("wv", [NL, NCC, 128, C], f32r, kind="ExternalInput")
    d_bqk = nc.dram_tensor("bqk", [NL, 128, 12], f32, kind="ExternalInput")
    d_apw = nc.dram_tensor("apw", [NL, NCC, 128, C], f32r, kind="ExternalInput")
    d_fcw = nc.dram_tensor("fcw", [NL, NFB, 128, C], f32r, kind="ExternalInput")
    d_fcb = nc.dram_tensor("fcb", [NL, 128, NFB], f32, kind="ExternalInput")
    d_prw = nc.dram_tensor("prw", [NL, NFB, 128, C], f32r, kind="ExternalInput")
    d_lmw = nc.dram_tensor("lmw", [NN, 128, NCC * NW], f32r, kind="ExternalInput")
    d_mask = nc.dram_tensor("mask", [NCH, 128, 256], f32r, kind="ExternalInput")
    d_bias = nc.dram_tensor("brows", [NL, 3, C], f32r, kind="ExternalInput")
    d_lmb = nc.dram_tensor("lmb", [1, VSH], f32r, kind="ExternalInput")

    d_cid = nc.dram_tensor("cident", [128, 128], f32r, kind="ExternalInput")
    d_con = nc.dram_tensor("cones", [128, 128], f32r, kind="ExternalInput")

    d_logits = nc.dram_tensor("logits", [T, VSH], f32, kind="ExternalOutput")
    d_expsum = nc.dram_tensor("expsum", [T, 1], f32, kind="ExternalOutput")

    with tile.TileContext(nc) as tc:
        with (
            tc.tile_pool(name="const", bufs=1) as cpool,
            tc.tile_pool(name="act", bufs=1) as apool,
            tc.tile_pool(name="act2", bufs=2) as a2pool,
            tc.tile_pool(name="wstream", bufs=2) as wpool,
            tc.tile_pool(name="psum", bufs=2, space="PSUM") as pp,
            tc.tile_pool(name="dram", bufs=1, space="DRAM") as dpool,
        ):
            ident = cpool.tile([128, 128], f32r, tag="ident")
            nc.sync.dma_start(ident[:], d_cid.ap())
            cones = cpool.tile([128, 128], f32r, tag="cones")
            nc.sync.dma_start(cones[:], d_con.ap())
            ones1 = cones[0:1, 0:128]

            masks = []
            for s in range(NCH):
                m = cpool.tile([128, 256], f32r, tag=f"mask{s}")
                nc.sync.dma_start(m[:], d_mask.ap()[s])
                masks.append(m)

            # persistent residual stream, token-major [2][128, C] fp32
            xs = []
            for i in range(2):
                xt = cpool.tile([128, C], f32, tag=f"x{i}")
                nc.sync.dma_start(xt[:], d_x0.ap()[i])
                xs.append(xt)

            def layernorm(xt, htag):
                """pure (x-mu)*rstd -> new f32r tile [128, C]"""
                stats = apool.tile([128, 12], f32, tag="ln_st")
                nc.vector.bn_stats(stats[:, 0:6], xt[:, 0:384])
                nc.vector.bn_stats(stats[:, 6:12], xt[:, 384:768])
                mv = apool.tile([128, 2], f32, tag="ln_mv")
                nc.vector.bn_aggr(mv[:], stats[:])
                ve = apool.tile([128, 1], f32, tag="ln_ve")
                nc.vector.tensor_scalar_add(ve[:], mv[:, 1:2], EPS)
                std = apool.tile([128, 1], f32, tag="ln_std")
                nc.scalar.activation(std[:], ve[:], AF.Sqrt)
                rstd = apool.tile([128, 1], f32, tag="ln_rstd")
                nc.vector.reciprocal(rstd[:], std[:])
                mn = apool.tile([128, 1], f32, tag="ln_mn")
                nc.vector.tensor_scalar(mn[:], mv[:, 0:1], rstd[:], -1.0,
                                        ALU.mult, ALU.mult)
                ht = apool.tile([128, C], f32r, tag=htag)
                nc.vector.tensor_scalar(ht[:], xt[:], rstd[:], mn[:],
                                        ALU.mult, ALU.add)
                return ht

            def transpose_pair(h0, h1, ttag):
                """[2][128, C] f32r -> 6 tiles [128, 256] f32r (feature-major)"""
                out = []
                for c in range(NCC):
                    dst = a2pool.tile([128, 256], f32r, tag=f"{ttag}{c}")
                    for i, h in enumerate((h0, h1)):
                        ps = pp.tile([128, 128], f32r, tag="ps_tr")
                        nc.tensor.matmul(ps[:], h[:, c * 128:(c + 1) * 128],
                                         ident[:], is_transpose=True)
                        nc.vector.tensor_copy(dst[:, i * 128:(i + 1) * 128], ps[:])
                    out.append(dst)
                return out

            def row_bias(ps, brow_ap):
                """add a [1, C] bias row into a [128, C] psum via K=1 matmul"""
                for n0, nw_ in ((0, 512), (512, 256)):
                    nc.tensor.matmul(ps[:, n0:n0 + nw_], ones1,
                                     brow_ap[:, n0:n0 + nw_],
                                     start=False, stop=True)

            for l in range(NL):
                # ---- LN1 + transpose ----
                h0 = layernorm(xs[0], "h0")
                h1 = layernorm(xs[1], "h1")
                hT = transpose_pair(h0, h1, "hT")

                bqk = apool.tile([128, 12], f32, tag="bqk")
                nc.sync.dma_start(bqk[:], d_bqk.ap()[l])
                brow = None
                if use_bias:
                    brow = apool.tile([3, C], f32r, tag="brow")
                    nc.sync.dma_start(brow[:], d_bias.ap()[l])

                # ---- kT (own tokens, feature-major) ----
                kT_own = []
                for m in range(NCC):
                    wt = wpool.tile([128, C], f32r, tag="w_a", bufs=4)
                    nc.sync.dma_start(wt[:], d_wk.ap()[l, m])
                    ps = pp.tile([128, 256], f32, tag="ps_med")
                    for c in range(NCC):
                        nc.tensor.matmul(ps[:], wt[:, c * 128:(c + 1) * 128],
                                         hT[c][:, 0:256],
                                         start=(c == 0), stop=(c == NCC - 1))
                    kt = apool.tile([128, 256], f32r, tag=f"kTo{m}")
                    nc.vector.tensor_scalar(kt[:], ps[:], bqk[:, 6 + m:7 + m],
                                            None, ALU.add)
                    kT_own.append(kt)

                # ---- v (own tokens, token-major) ----
                ps_v = [pp.tile([128, C], f32, tag="ps_big", name=f"ps_v{l}_{i}")
                        for i in range(2)]
                for c in range(NCC):
                    wt = wpool.tile([128, C], f32r, tag="w_a", bufs=4)
                    nc.sync.dma_start(wt[:], d_wv.ap()[l, c])
                    for qi in range(2):
                        for n0, nw_ in ((0, 512), (512, 256)):
                            nc.tensor.matmul(
                                ps_v[qi][:, n0:n0 + nw_],
                                hT[c][:, qi * 128:(qi + 1) * 128],
                                wt[:, n0:n0 + nw_],
                                start=(c == 0),
                                stop=(c == NCC - 1 and not use_bias))
                v_own = []
                for qi in range(2):
                    if use_bias:
                        row_bias(ps_v[qi], brow[0:1, :])
                    vt = apool.tile([128, C], f32r, tag=f"vo{qi}")
                    nc.vector.tensor_copy(vt[:], ps_v[qi][:])
                    v_own.append(vt)

                # ---- bounce out + AllGather(kT_own, v_own) ----
                kvb_in = dpool.tile([KVE], f32r, tag="kvb_in")
                kvb_out = dpool.tile([4 * KVE], f32r, tag="kvb_out")
                for m in range(NCC):
                    dst = kvb_in[m * 128 * 256:(m + 1) * 128 * 256]
                    nc.sync.dma_start(
                        dst.rearrange("(p n) -> p n", p=128), kT_own[m][:])
                for qi in range(2):
                    dst = kvb_in[KTE + qi * 128 * C: KTE + (qi + 1) * 128 * C]
                    nc.sync.dma_start(
                        dst.rearrange("(p n) -> p n", p=128), v_own[qi][:])
                nc.gpsimd.collective_compute(
                    "AllGather", ALU.bypass,
                    replica_groups=[[0, 1, 2, 3], [4, 5, 6, 7]],
                    ins=[kvb_in[:]], outs=[kvb_out[:]])

                # ---- qT (own tokens), overlaps the AG ----
                qT = []
                for m in range(NCC):
                    wt = wpool.tile([128, C], f32r, tag="w_a", bufs=4)
                    nc.sync.dma_start(wt[:], d_wq.ap()[l, m])
                    ps = pp.tile([128, 256], f32, tag="ps_med")
                    for c in range(NCC):
                        nc.tensor.matmul(ps[:], wt[:, c * 128:(c + 1) * 128],
                                         hT[c][:, 0:256],
                                         start=(c == 0), stop=(c == NCC - 1))
                    qt = apool.tile([128, 256], f32r, tag=f"qT{m}")
                    nc.vector.tensor_scalar(qt[:], ps[:], bqk[:, m:m + 1],
                                            None, ALU.add)
                    qT.append(qt)

                # ---- load gathered kT_full / v_full, build v_aug ----
                kT_full = []
                for c in range(NCC):
                    kf = apool.tile([128, T], f32r, tag=f"kTf{c}")
                    for j in range(4):
                        src = kvb_out[j * KVE + c * 128 * 256:
                                      j * KVE + (c + 1) * 128 * 256]
                        nc.sync.dma_start(
                            kf[:, j * 256:(j + 1) * 256],
                            src.rearrange("(p n) -> p n", p=128))
                    kT_full.append(kf)
                v_aug = []
                for s in range(NCH):
                    j, i = s // 2, s % 2
                    vf = a2pool.tile([128, C], f32r, tag="vf")
                    src = kvb_out[j * KVE + KTE + i * 128 * C:
                                  j * KVE + KTE + (i + 1) * 128 * C]
                    nc.sync.dma_start(vf[:], src.rearrange("(p n) -> p n", p=128))
                    va = apool.tile([128, H * 66], f32r, tag=f"va{s}")
                    vav = va[:].rearrange("p (h e) -> p h e", e=66)
                    nc.vector.tensor_copy(
                        vav[:, :, 0:64],
                        vf[:].rearrange("p (h e) -> p h e", e=64))
                    nc.vector.tensor_copy(
                        vav[:, :, 64:66],
                        cones[:, 0:24].rearrange("p (h e) -> p h e", e=2))
                    v_aug.append(va)

                # ---- attention ----
                att0 = apool.tile([128, C], f32r, tag="att0")
                att1 = apool.tile([128, C], f32r, tag="att1")
                for h in range(H):
                    ct, r0 = h // 2, (h % 2) * 64
                    expT = []
                    for s in range(NCH):
                        ps = pp.tile([128, 256], f32, tag="ps_med")
                        nc.tensor.matmul(
                            ps[:], kT_full[ct][r0:r0 + 64, s * 128:(s + 1) * 128],
                            qT[ct][r0:r0 + 64, 0:256])
                        et = a2pool.tile([128, 256], f32r, tag=f"expT{s}")
                        nc.scalar.activation(et[:], ps[:], AF.Exp, scale=SCALE)
                        nc.vector.tensor_mul(et[:], et[:], masks[s][:])
                        expT.append(et)
                    for qi, att in enumerate((att0, att1)):
                        aps = pp.tile([128, 66], f32, tag="ps_big")
                        for s in range(NCH):
                            nc.tensor.matmul(
                                aps[:], expT[s][:, qi * 128:(qi + 1) * 128],
                                v_aug[s][:, h * 66:(h + 1) * 66],
                                start=(s == 0), stop=(s == NCH - 1))
                        rec = apool.tile([128, 1], f32, tag="rec")
                        nc.vector.reciprocal(rec[:], aps[:, 64:65])
                        nc.vector.tensor_scalar(
                            att[:, h * 64:(h + 1) * 64], aps[:, 0:64],
                            rec[:], None, ALU.mult)

                # ---- attnT + proj + residual ----
                attT = transpose_pair(att0, att1, "hT")
                ps_p = [pp.tile([128, C], f32, tag="ps_big", name=f"ps_p{l}_{i}")
                        for i in range(2)]
                for c in range(NCC):
                    wt = wpool.tile([128, C], f32r, tag="w_a", bufs=4)
                    nc.sync.dma_start(wt[:], d_apw.ap()[l, c])
                    for qi in range(2):
                        for n0, nw_ in ((0, 512), (512, 256)):
                            nc.tensor.matmul(
                                ps_p[qi][:, n0:n0 + nw_],
                                attT[c][:, qi * 128:(qi + 1) * 128],
                                wt[:, n0:n0 + nw_],
                                start=(c == 0),
                                stop=(c == NCC - 1 and not use_bias))
                for qi in range(2):
                    if use_bias:
                        row_bias(ps_p[qi], brow[1:2, :])
                    nc.vector.tensor_add(xs[qi][:], xs[qi][:], ps_p[qi][:])

                # ---- LN2 + FFN ----
                g0 = layernorm(xs[0], "h0")
                g1 = layernorm(xs[1], "h1")
                h2T = transpose_pair(g0, g1, "hT")
                fcb = apool.tile([128, NFB], f32, tag="fcb")
                nc.sync.dma_start(fcb[:], d_fcb.ap()[l])
                relu = []
                for fb in range(NFB):
                    wt = wpool.tile([128, C], f32r, tag="w_fc")
                    nc.sync.dma_start(wt[:], d_fcw.ap()[l, fb])
                    ps = pp.tile([128, 256], f32, tag="ps_med")
                    for c in range(NCC):
                        nc.tensor.matmul(ps[:], wt[:, c * 128:(c + 1) * 128],
                                         h2T[c][:, 0:256],
                                         start=(c == 0), stop=(c == NCC - 1))
                    rt = a2pool.tile([128, 256], f32r, tag=f"relu{fb % 8}", bufs=1)
                    nc.scalar.activation(rt[:], ps[:], AF.Relu,
                                         bias=fcb[:, fb:fb + 1])
                    relu.append(rt)
                ps_r = [pp.tile([128, C], f32, tag="ps_big", name=f"ps_r{l}_{i}")
                        for i in range(2)]
                for fb in range(NFB):
                    wt = wpool.tile([128, C], f32r, tag="w_pr")
                    nc.sync.dma_start(wt[:], d_prw.ap()[l, fb])
                    for qi in range(2):
                        for n0, nw_ in ((0, 512), (512, 256)):
                            nc.tensor.matmul(
                                ps_r[qi][:, n0:n0 + nw_],
                                relu[fb][:, qi * 128:(qi + 1) * 128],
                                wt[:, n0:n0 + nw_],
                                start=(fb == 0),
                                stop=(fb == NFB - 1 and not use_bias))
                for qi in range(2):
                    if use_bias:
                        row_bias(ps_r[qi], brow[2:3, :])
                    nc.vector.tensor_add(xs[qi][:], xs[qi][:], ps_r[qi][:])

            # ---- final LN + transpose + AllGather ----
            f0 = layernorm(xs[0], "h0")
            f1 = layernorm(xs[1], "h1")
            hfT = transpose_pair(f0, f1, "hT")
            hb_in = dpool.tile([KTE], f32r, tag="hb_in")
            hb_out = dpool.tile([4 * KTE], f32r, tag="hb_out")
            for c in range(NCC):
                dst = hb_in[c * 128 * 256:(c + 1) * 128 * 256]
                nc.sync.dma_start(dst.rearrange("(p n) -> p n", p=128), hfT[c][:])
            nc.gpsimd.collective_compute(
                "AllGather", ALU.bypass,
                replica_groups=[[0, 1, 2, 3], [4, 5, 6, 7]],
                ins=[hb_in[:]], outs=[hb_out[:]])
            hT_full = []
            for c in range(NCC):
                hf = apool.tile([128, T], f32r, tag=f"kTf{c}")
                for j in range(4):
                    src = hb_out[j * KTE + c * 128 * 256:
                                 j * KTE + (c + 1) * 128 * 256]
                    nc.sync.dma_start(hf[:, j * 256:(j + 1) * 256],
                                      src.rearrange("(p n) -> p n", p=128))
                hT_full.append(hf)

            # ---- lm_head ----
            lmb = None
            if use_bias:
                lmb = apool.tile([1, VSH], f32r, tag="lmb")
                nc.sync.dma_start(lmb[:], d_lmb.ap())
            scr = apool.tile([128, NW], f32, tag="lm_scr")
            sums = [apool.tile([128, NN], f32, tag=f"sums{s}", name=f"sums{s}")
                    for s in range(NCH)]
            for ni in range(NN):
                wt = wpool.tile([128, NCC * NW], f32r, tag="w_lm")
                nc.sync.dma_start(wt[:], d_lmw.ap()[ni])
                for s in range(NCH):
                    ps = pp.tile([128, NW], f32, tag="ps_med")
                    for c in range(NCC):
                        nc.tensor.matmul(ps[:],
                                         hT_full[c][:, s * 128:(s + 1) * 128],
                                         wt[:, c * NW:(c + 1) * NW],
                                         start=(c == 0),
                                         stop=(c == NCC - 1 and not use_bias))
                    if use_bias:
                        nc.tensor.matmul(ps[:], ones1,
                                         lmb[0:1, ni * NW:(ni + 1) * NW],
                                         start=False, stop=True)
                    lt = a2pool.tile([128, NW], f32, tag="lm_out")
                    nc.vector.tensor_copy(lt[:], ps[:])
                    nc.sync.dma_start(
                        d_logits.ap()[TRUE[s] * 128:(TRUE[s] + 1) * 128,
                                      ni * NW:(ni + 1) * NW], lt[:])
                    nc.scalar.activation(scr[:], ps[:], AF.Exp,
                                         accum_out=sums[s][:, ni:ni + 1])
            for s in range(NCH):
                es = apool.tile([128, 1], f32, tag="es")
                nc.vector.tensor_reduce(es[:], sums[s][:],
                                        axis=mybir.AxisListType.X, op=ALU.add)
                nc.sync.dma_start(
                    d_expsum.ap()[TRUE[s] * 128:(TRUE[s] + 1) * 128, :], es[:])

    nc.compile()
    return nc


def _fold_col_tiles(w, nb):
    """[C, nb*128] -> [nb, 128, C] with out[b, p, c*128+f] = w[c*128+p, b*128+f]"""
    cc = w.shape[0] // 128
    return np.ascontiguousarray(
        w.reshape(cc, 128, nb, 128).transpose(2, 1, 0, 3).reshape(nb, 128, cc * 128))


def _prep(inputs, nlayer):
    """Host-side weight folding/repacking. Returns per-core in_maps."""
    g = {}
    for k, v in inputs.items():
        a = np.asarray(v)
        g[k] = a if a.dtype in (np.int64, np.int32) else a.astype(np.float32)
    idx, targets = g["idx"], g["targets"]
    x0 = g["wte"][idx] + g["wpe"][:T][None, :, :]           # [B, T, C] f32

    wq_t = np.empty((nlayer, NCC, 128, C), np.float32)
    wk_t = np.empty((nlayer, NCC, 128, C), np.float32)
    wv_t = np.empty((nlayer, NCC, 128, C), np.float32)
    apw_t = np.empty((nlayer, NCC, 128, C), np.float32)
    fcw_t = np.empty((nlayer, NFB, 128, C), np.float32)
    prw_t = np.empty((nlayer, NFB, 128, C), np.float32)
    bqk = np.zeros((nlayer, 128, 12), np.float32)
    fcb = np.zeros((nlayer, 128, NFB), np.float32)
    brows = np.zeros((nlayer, 3, C), np.float32)
    for l in range(nlayer):
        wq = g["ln1_g"][l][:, None] * g["wq"][l]
        wk = g["ln1_g"][l][:, None] * g["wk"][l]
        wv = g["ln1_g"][l][:, None] * g["wv"][l]
        fw = g["ln2_g"][l][:, None] * g["fc_w"][l]
        fb = g["fc_b"][l] + g["ln2_b"][l] @ g["fc_w"][l]
        wq_t[l] = _fold_col_tiles(wq, NCC)
        wk_t[l] = _fold_col_tiles(wk, NCC)
        wv_t[l] = wv.reshape(NCC, 128, C)
        apw_t[l] = g["attn_pw"][l].reshape(NCC, 128, C)
        fcw_t[l] = _fold_col_tiles(fw, NFB)
        prw_t[l] = g["pr_w"][l].reshape(NFB, 128, C)
        bqk[l, :, 0:6] = (g["ln1_b"][l] @ g["wq"][l]).reshape(6, 128).T
        bqk[l, :, 6:12] = (g["ln1_b"][l] @ g["wk"][l]).reshape(6, 128).T
        fcb[l] = fb.reshape(NFB, 128).T
        brows[l, 0] = g["ln1_b"][l] @ g["wv"][l]
        brows[l, 1] = g["attn_pb"][l]
        brows[l, 2] = g["pr_b"][l]

    lmw = g["lnf_g"][:, None] * g["lm_w"]                   # [C, V]
    lmb_full = g["lnf_b"] @ g["lm_w"]                       # [V]
    lmw_pad = np.zeros((C, VPAD), np.float32)
    lmw_pad[:, :V] = lmw
    lmb_pad = np.zeros((VPAD,), np.float32)
    lmb_pad[:V] = lmb_full

    use_bias = bool(np.any(brows != 0) or np.any(lmb_pad != 0))

    shared = dict(wq=wq_t, wk=wk_t, wv=wv_t, apw=apw_t, fcw=fcw_t, prw=prw_t,
                  bqk=bqk, fcb=fcb, brows=brows)

    per_core = []
    for core in range(8):
        grp, r = core // 4, core % 4
        oc = [r, 7 - r]
        x0_own = np.concatenate(
            [x0[grp, c * 128:(c + 1) * 128] for c in oc], axis=0)
        mask = np.zeros((NCH, 128, 256), np.float32)
        for s in range(NCH):
            for qi, cq in enumerate(oc):
                ck = TRUE[s]
                if ck < cq:
                    mask[s, :, qi * 128:(qi + 1) * 128] = 1.0
                elif ck == cq:
                    tri = (np.arange(128)[None, :] >= np.arange(128)[:, None])
                    mask[s, :, qi * 128:(qi + 1) * 128] = tri.astype(np.float32)
        sh = lmw_pad[:, r * VSH:(r + 1) * VSH]
        lmw_core = np.ascontiguousarray(
            sh.reshape(NCC, 128, NN, NW).transpose(2, 1, 0, 3)
            .reshape(NN, 128, NCC * NW))
        per_core.append(dict(
            cident=np.eye(128, dtype=np.float32),
            cones=np.ones((128, 128), np.float32),
            x0=np.ascontiguousarray(x0_own.reshape(2, 128, C)),
            mask=mask, lmw=lmw_core,
            lmb=lmb_pad[r * VSH:(r + 1) * VSH].reshape(1, VSH), **shared))
    return per_core, idx, targets, use_bias


def assemble(results, idx, targets):
    logits = np.empty((B, T, V), np.float32)
    expsum = np.zeros((B, T), np.float64)
    npad_last = VPAD - V                                    # pads in rank-3 shard
    for core in range(8):
        grp, r = core // 4, core % 4
        sh = results[core]["logits"]                        # [T, VSH]
        lo = r * VSH
        hi = min((r + 1) * VSH, V)
        if lo < V:
            logits[grp, :, lo:hi] = sh[:, :hi - lo]
        es = results[core]["expsum"][:, 0].astype(np.float64)
        if r == 3:
            es = es - npad_last                             # pad cols give exp(0)=1
        expsum[grp] += es
    lse = np.log(expsum)                                    # [B, T]
    tl = np.take_along_axis(logits, np.asarray(targets)[..., None], axis=-1)[..., 0]
    loss = np.float32(np.mean(lse - tl.astype(np.float64)))
    return logits, loss


def run(inputs, trace=False):
    nlayer = _NLAYER
    per_core, idx, targets, use_bias = _prep(inputs, nlayer)
    key = (nlayer, use_bias)
    if key not in _cache:
        _cache[key] = _build(nlayer, use_bias)
    nc = _cache[key]
    res = run_bass_kernel_spmd(nc, per_core, core_ids=list(range(8)),
                               trace=trace)
    out = assemble(res.results, idx, targets)
    return out, res


def kernel(**inputs):
    out, _ = run(inputs, trace=False)
    return out


# revision 13
# speedup vs baseline: 1.2830x; 1.2830x over previous
"""GPT-2-style forward pass on 8 Trainium2 NeuronCores.

Sharding: DP=2 over batch x SP=4 over token chunks (interleaved {r, 7-r}
per rank for causal load balance). Per layer, the k/v activations are
AllGathered (bf16, split into two collectives for compute overlap) within
each 4-core group; vocab-sharded lm_head after a final AllGather.
Matmuls run in float32r except the attention core (bf16); the residual
stream stays fp32. LN gains/biases are folded into adjacent matmul
weights on the host (exact). The SPMD program is rank-uniform: causal
structure is expressed via per-rank 0/1 mask data.
"""
import sys

sys.path.insert(0, "/opt/trn_rl_repo")

import ml_dtypes
import numpy as np
import concourse.bass as bass
import concourse.mybir as mybir
import concourse.bacc as bacc
import concourse.tile as tile
from concourse.bass_utils import run_bass_kernel_spmd

f32 = mybir.dt.float32
f32r = mybir.dt.float32r
bf16 = mybir.dt.bfloat16
AF = mybir.ActivationFunctionType
ALU = mybir.AluOpType

V, S, C, H, L = 50257, 1024, 768, 12, 6
FF, HD = 3072, 64
B, T = 2, 1024
NCH = 8                   # token chunks per batch element
NCC = C // 128            # 6 contraction chunks over C
NFB = FF // 128           # 24 FFN blocks
NW = 512                  # lm_head vocab tile width
NN = 25                   # lm_head vocab tiles per rank
VSH = NW * NN             # 12800 vocab shard per rank
VPAD = 4 * VSH            # 51200
SCALE = float(C) ** -0.5
EPS = 1e-5
TRUE = [0, 7, 1, 6, 2, 5, 3, 4]  # gather slot -> true chunk id
KTE = C * 256             # elems of one rank's kT (768x256)
VE = 256 * C              # elems of one rank's v (256x768)
RG = [[0, 1, 2, 3], [4, 5, 6, 7]]

_NLAYER = L               # dev override hook
_cache = {}


def _build(nlayer, use_bias):
    nc = bacc.Bacc("TRN2", target_bir_lowering=False, debug=False,
                   enable_asserts=False, num_devices=8)
    NL = nlayer

    d_x0 = nc.dram_tensor("x0", [2, 128, C], f32, kind="ExternalInput")
    d_wq = nc.dram_tensor("wq", [NL, NCC, 128, C], f32r, kind="ExternalInput")
    d_wk = nc.dram_tensor("wk", [NL, NCC, 128, C], f32r, kind="ExternalInput")
    d_wv = nc.dram_tensor("wv", [NL, NCC, 128, C], f32r, kind="ExternalInput")
    d_bqk = nc.dram_tensor("bqk", [NL, 128, 12], f32, kind="ExternalInput")
    d_apw = nc.dram_tensor("apw", [NL, NCC, 128, C], f32r, kind="ExternalInput")
    d_fcw = nc.dram_tensor("fcw", [NL, NFB, 128, C], f32r, kind="ExternalInput")
    d_fcb = nc.dram_tensor("fcb", [NL, 128, NFB], f32, kind="ExternalInput")
    d_prw = nc.dram_tensor("prw", [NL, NFB, 128, C], f32r, kind="ExternalInput")
    d_lmw = nc.dram_tensor("lmw", [NN, 128, NCC * NW], f32r, kind="ExternalInput")
    d_mask = nc.dram_tensor("mask", [NCH, 128, 256], bf16, kind="ExternalInput")
    d_bias = nc.dram_tensor("brows", [NL, 3, C], f32r, kind="ExternalInput")
    d_lmb = nc.dram_tensor("lmb", [1, VSH], f32r, kind="ExternalInput")
    d_cid = nc.dram_tensor("cident", [128, 128], f32r, kind="ExternalInput")
    d_con = nc.dram_tensor("cones", [128, 128], f32r, kind="ExternalInput")
    d_cob = nc.dram_tensor("conesb", [128, 24], bf16, kind="ExternalInput")

    d_logits = nc.dram_tensor("logits", [T, VSH], f32, kind="ExternalOutput")
    d_expsum = nc.dram_tensor("expsum", [T, 1], f32, kind="ExternalOutput")

    with tile.TileContext(nc) as tc:
        with (
            tc.tile_pool(name="const", bufs=1) as cpool,
            tc.tile_pool(name="act", bufs=1) as apool,
            tc.tile_pool(name="act2", bufs=2) as a2pool,
            tc.tile_pool(name="wstream", bufs=2) as wpool,
            tc.tile_pool(name="psum", bufs=2, space="PSUM") as pp,
            tc.tile_pool(name="dram", bufs=1, space="DRAM") as dpool,
        ):
            ident = cpool.tile([128, 128], f32r, tag="ident")
            nc.sync.dma_start(ident[:], d_cid.ap())
            cones = cpool.tile([128, 128], f32r, tag="cones")
            nc.sync.dma_start(cones[:], d_con.ap())
            conesb = cpool.tile([128, 24], bf16, tag="conesb")
            nc.sync.dma_start(conesb[:], d_cob.ap())
            ones1 = cones[0:1, 0:128]

            masks = []
            for s in range(NCH):
                m = cpool.tile([128, 256], bf16, tag=f"mask{s}")
                nc.sync.dma_start(m[:], d_mask.ap()[s])
                masks.append(m)

            # persistent residual stream, token-major [2][128, C] fp32
            xs = []
            for i in range(2):
                xt = cpool.tile([128, C], f32, tag=f"x{i}")
                nc.sync.dma_start(xt[:], d_x0.ap()[i])
                xs.append(xt)

            def layernorm(xt, htag):
                """pure (x-mu)*rstd -> new f32r tile [128, C]"""
                stats = apool.tile([128, 12], f32, tag="ln_st")
                nc.vector.bn_stats(stats[:, 0:6], xt[:, 0:384])
                nc.vector.bn_stats(stats[:, 6:12], xt[:, 384:768])
                mv = apool.tile([128, 2], f32, tag="ln_mv")
                nc.vector.bn_aggr(mv[:], stats[:])
                ve = apool.tile([128, 1], f32, tag="ln_ve")
                nc.vector.tensor_scalar_add(ve[:], mv[:, 1:2], EPS)
                std = apool.tile([128, 1], f32, tag="ln_std")
                nc.scalar.activation(std[:], ve[:], AF.Sqrt)
                rstd = apool.tile([128, 1], f32, tag="ln_rstd")
                nc.vector.reciprocal(rstd[:], std[:])
                mn = apool.tile([128, 1], f32, tag="ln_mn")
                nc.vector.tensor_scalar(mn[:], mv[:, 0:1], rstd[:], -1.0,
                                        ALU.mult, ALU.mult)
                ht = apool.tile([128, C], f32r, tag=htag)
                nc.vector.tensor_scalar(ht[:], xt[:], rstd[:], mn[:],
                                        ALU.mult, ALU.add)
                return ht

            def transpose_pair(h0, h1, ttag):
                """[2][128, C] f32r -> 6 tiles [128, 256] f32r (feature-major)"""
                out = []
                for c in range(NCC):
                    dst = a2pool.tile([128, 256], f32r, tag=f"{ttag}{c}")
                    for i, h in enumerate((h0, h1)):
                        ps = pp.tile([128, 128], f32r, tag="ps_tr")
                        nc.tensor.matmul(ps[:], h[:, c * 128:(c + 1) * 128],
                                         ident[:], is_transpose=True)
                        nc.vector.tensor_copy(dst[:, i * 128:(i + 1) * 128], ps[:])
                    out.append(dst)
                return out

            def row_bias(ps, brow_ap):
                """add a [1, C] bias row into a [128, C] psum via K=1 matmul"""
                for n0, nw_ in ((0, 512), (512, 256)):
                    nc.tensor.matmul(ps[:, n0:n0 + nw_], ones1,
                                     brow_ap[:, n0:n0 + nw_],
                                     start=False, stop=True)

            for l in range(NL):
                # ---- LN1 + transpose ----
                h0 = layernorm(xs[0], "h0")
                h1 = layernorm(xs[1], "h1")
                hT = transpose_pair(h0, h1, "hT")

                bqk = apool.tile([128, 12], f32, tag="bqk")
                nc.sync.dma_start(bqk[:], d_bqk.ap()[l])
                brow = None
                if use_bias:
                    brow = apool.tile([3, C], f32r, tag="brow")
                    nc.sync.dma_start(brow[:], d_bias.ap()[l])

                # ---- kT (own tokens, feature-major, bf16) + AG(k) asap ----
                kb_in = dpool.tile([KTE], bf16, tag="kb_in")
                kb_out = dpool.tile([4 * KTE], bf16, tag="kb_out")
                for m in range(NCC):
                    wt = wpool.tile([128, C], f32r, tag="w_a", bufs=4)
                    nc.sync.dma_start(wt[:], d_wk.ap()[l, m])
                    ps = pp.tile([128, 256], f32, tag="ps_med")
                    for c in range(NCC):
                        nc.tensor.matmul(ps[:], wt[:, c * 128:(c + 1) * 128],
                                         hT[c][:, 0:256],
                                         start=(c == 0), stop=(c == NCC - 1))
                    kt = apool.tile([128, 256], bf16, tag=f"kTo{m}")
                    nc.vector.tensor_scalar(kt[:], ps[:], bqk[:, 6 + m:7 + m],
                                            None, ALU.add)
                    dst = kb_in[m * 128 * 256:(m + 1) * 128 * 256]
                    nc.sync.dma_start(dst.rearrange("(p n) -> p n", p=128), kt[:])
                nc.gpsimd.collective_compute(
                    "AllGather", ALU.bypass, replica_groups=RG,
                    ins=[kb_in[:]], outs=[kb_out[:]])

                # ---- v (own tokens, token-major, bf16) + AG(v) ----
                vb_in = dpool.tile([VE], bf16, tag="vb_in")
                vb_out = dpool.tile([4 * VE], bf16, tag="vb_out")
                ps_v = [pp.tile([128, C], f32, tag="ps_big", name=f"ps_v{l}_{i}")
                        for i in range(2)]
                for c in range(NCC):
                    wt = wpool.tile([128, C], f32r, tag="w_a", bufs=4)
                    nc.sync.dma_start(wt[:], d_wv.ap()[l, c])
                    for qi in range(2):
                        for n0, nw_ in ((0, 512), (512, 256)):
                            nc.tensor.matmul(
                                ps_v[qi][:, n0:n0 + nw_],
                                hT[c][:, qi * 128:(qi + 1) * 128],
                                wt[:, n0:n0 + nw_],
                                start=(c == 0),
                                stop=(c == NCC - 1 and not use_bias))
                for qi in range(2):
                    if use_bias:
                        row_bias(ps_v[qi], brow[0:1, :])
                    vt = apool.tile([128, C], bf16, tag=f"vo{qi}")
                    nc.vector.tensor_copy(vt[:], ps_v[qi][:])
                    dst = vb_in[qi * 128 * C:(qi + 1) * 128 * C]
                    nc.sync.dma_start(dst.rearrange("(p n) -> p n", p=128), vt[:])
                nc.gpsimd.collective_compute(
                    "AllGather", ALU.bypass, replica_groups=RG,
                    ins=[vb_in[:]], outs=[vb_out[:]])

                # ---- qT (own tokens, bf16), overlaps the AGs ----
                qT = []
                for m in range(NCC):
                    wt = wpool.tile([128, C], f32r, tag="w_a", bufs=4)
                    nc.sync.dma_start(wt[:], d_wq.ap()[l, m])
                    ps = pp.tile([128, 256], f32, tag="ps_med")
                    for c in range(NCC):
                        nc.tensor.matmul(ps[:], wt[:, c * 128:(c + 1) * 128],
                                         hT[c][:, 0:256],
                                         start=(c == 0), stop=(c == NCC - 1))
                    qt = apool.tile([128, 256], bf16, tag=f"qT{m}")
                    nc.vector.tensor_scalar(qt[:], ps[:], bqk[:, m:m + 1],
                                            None, ALU.add)
                    qT.append(qt)

                # ---- load gathered kT_full / v_full, build v_aug ----
                kT_full = []
                for c in range(NCC):
                    kf = apool.tile([128, T], bf16, tag=f"kTf{c}")
                    for j in range(4):
                        src = kb_out[j * KTE + c * 128 * 256:
                                     j * KTE + (c + 1) * 128 * 256]
                        nc.sync.dma_start(
                            kf[:, j * 256:(j + 1) * 256],
                            src.rearrange("(p n) -> p n", p=128))
                    kT_full.append(kf)
                v_aug = []
                for s in range(NCH):
                    j, i = s // 2, s % 2
                    vf = a2pool.tile([128, C], bf16, tag="vf")
                    src = vb_out[j * VE + i * 128 * C:
                                 j * VE + (i + 1) * 128 * C]
                    nc.sync.dma_start(vf[:], src.rearrange("(p n) -> p n", p=128))
                    va = apool.tile([128, H * 66], bf16, tag=f"va{s}")
                    vav = va[:].rearrange("p (h e) -> p h e", e=66)
                    nc.vector.tensor_copy(
                        vav[:, :, 0:64],
                        vf[:].rearrange("p (h e) -> p h e", e=64))
                    nc.vector.tensor_copy(
                        vav[:, :, 64:66],
                        conesb[:, 0:24].rearrange("p (h e) -> p h e", e=2))
                    v_aug.append(va)

                # ---- attention ----
                att0 = apool.tile([128, C], f32r, tag="att0")
                att1 = apool.tile([128, C], f32r, tag="att1")
                for h in range(H):
                    ct, r0 = h // 2, (h % 2) * 64
                    expT = []
                    for s in range(NCH):
                        ps = pp.tile([128, 256], f32, tag="ps_med")
                        nc.tensor.matmul(
                            ps[:], kT_full[ct][r0:r0 + 64, s * 128:(s + 1) * 128],
                            qT[ct][r0:r0 + 64, 0:256])
                        et = a2pool.tile([128, 256], bf16, tag=f"expT{s}")
                        nc.scalar.activation(et[:], ps[:], AF.Exp, scale=SCALE)
                        nc.vector.tensor_mul(et[:], et[:], masks[s][:])
                        expT.append(et)
                    for qi, att in enumerate((att0, att1)):
                        aps = pp.tile([128, 66], f32, tag="ps_tr")
                        for s in range(NCH):
                            nc.tensor.matmul(
                                aps[:], expT[s][:, qi * 128:(qi + 1) * 128],
                                v_aug[s][:, h * 66:(h + 1) * 66],
                                start=(s == 0), stop=(s == NCH - 1))
                        rec = apool.tile([128, 1], f32, tag="rec")
                        nc.vector.reciprocal(rec[:], aps[:, 64:65])
                        nc.vector.tensor_scalar(
                            att[:, h * 64:(h + 1) * 64], aps[:, 0:64],
                            rec[:], None, ALU.mult)

                # ---- attnT + proj + residual ----
                attT = transpose_pair(att0, att1, "hT")
                ps_p = [pp.tile([128, C], f32, tag="ps_big", name=f"ps_p{l}_{i}")
                        for i in range(2)]
                for c in range(NCC):
                    wt = wpool.tile([128, C], f32r, tag="w_a", bufs=4)
                    nc.sync.dma_start(wt[:], d_apw.ap()[l, c])
                    for qi in range(2):
                        for n0, nw_ in ((0, 512), (512, 256)):
                            nc.tensor.matmul(
                                ps_p[qi][:, n0:n0 + nw_],
                                attT[c][:, qi * 128:(qi + 1) * 128],
                                wt[:, n0:n0 + nw_],
                                start=(c == 0),
                                stop=(c == NCC - 1 and not use_bias))
                for qi in range(2):
                    if use_bias:
                        row_bias(ps_p[qi], brow[1:2, :])
                    nc.vector.tensor_add(xs[qi][:], xs[qi][:], ps_p[qi][:])

                # ---- LN2 + FFN ----
                g0 = layernorm(xs[0], "h0")
                g1 = layernorm(xs[1], "h1")
                h2T = transpose_pair(g0, g1, "hT")
                fcb = apool.tile([128, NFB], f32, tag="fcb")
                nc.sync.dma_start(fcb[:], d_fcb.ap()[l])
                relu = []
                for fb in range(NFB):
                    wt = wpool.tile([128, C], f32r, tag="w_fc", bufs=4)
                    nc.sync.dma_start(wt[:], d_fcw.ap()[l, fb])
                    ps = pp.tile([128, 256], f32, tag="ps_med")
                    for c in range(NCC):
                        nc.tensor.matmul(ps[:], wt[:, c * 128:(c + 1) * 128],
                                         h2T[c][:, 0:256],
                                         start=(c == 0), stop=(c == NCC - 1))
                    rt = a2pool.tile([128, 256], f32r, tag=f"relu{fb % 8}",
                                     bufs=1)
                    nc.scalar.activation(rt[:], ps[:], AF.Relu,
                                         bias=fcb[:, fb:fb + 1])
                    relu.append(rt)
                ps_r = [pp.tile([128, C], f32, tag="ps_big", name=f"ps_r{l}_{i}")
                        for i in range(2)]
                for fb in range(NFB):
                    wt = wpool.tile([128, C], f32r, tag="w_pr", bufs=4)
                    nc.sync.dma_start(wt[:], d_prw.ap()[l, fb])
                    for qi in range(2):
                        for n0, nw_ in ((0, 512), (512, 256)):
                            nc.tensor.matmul(
                                ps_r[qi][:, n0:n0 + nw_],
                                relu[fb][:, qi * 128:(qi + 1) * 128],
                                wt[:, n0:n0 + nw_],
                                start=(fb == 0),
                                stop=(fb == NFB - 1 and not use_bias))
                for qi in range(2):
                    if use_bias:
                        row_bias(ps_r[qi], brow[2:3, :])
                    nc.vector.tensor_add(xs[qi][:], xs[qi][:], ps_r[qi][:])

            # ---- final LN + transpose + AllGather ----
            f0 = layernorm(xs[0], "h0")
            f1 = layernorm(xs[1], "h1")
            hfT = transpose_pair(f0, f1, "hT")
            hb_in = dpool.tile([KTE], f32r, tag="hb_in")
            hb_out = dpool.tile([4 * KTE], f32r, tag="hb_out")
            for c in range(NCC):
                dst = hb_in[c * 128 * 256:(c + 1) * 128 * 256]
                nc.sync.dma_start(dst.rearrange("(p n) -> p n", p=128), hfT[c][:])
            nc.gpsimd.collective_compute(
                "AllGather", ALU.bypass, replica_groups=RG,
                ins=[hb_in[:]], outs=[hb_out[:]])
            hT_full = []
            for c in range(NCC):
                hf = apool.tile([128, T], f32r, tag=f"hTf{c}")
                for j in range(4):
                    src = hb_out[j * KTE + c * 128 * 256:
                                 j * KTE + (c + 1) * 128 * 256]
                    nc.sync.dma_start(hf[:, j * 256:(j + 1) * 256],
                                      src.rearrange("(p n) -> p n", p=128))
                hT_full.append(hf)

            # ---- lm_head ----
            lmb = None
            if use_bias:
                lmb = apool.tile([1, VSH], f32r, tag="lmb")
                nc.sync.dma_start(lmb[:], d_lmb.ap())
            scr = apool.tile([128, NW], f32, tag="lm_scr")
            sums = [apool.tile([128, NN], f32, tag=f"sums{s}", name=f"sums{s}")
                    for s in range(NCH)]
            for ni in range(NN):
                wt = wpool.tile([128, NCC * NW], f32r, tag="w_lm", bufs=2)
                nc.sync.dma_start(wt[:], d_lmw.ap()[ni])
                for s in range(NCH):
                    ps = pp.tile([128, NW], f32, tag="ps_med")
                    for c in range(NCC):
                        nc.tensor.matmul(ps[:],
                                         hT_full[c][:, s * 128:(s + 1) * 128],
                                         wt[:, c * NW:(c + 1) * NW],
                                         start=(c == 0),
                                         stop=(c == NCC - 1 and not use_bias))
                    if use_bias:
                        nc.tensor.matmul(ps[:], ones1,
                                         lmb[0:1, ni * NW:(ni + 1) * NW],
                                         start=False, stop=True)
                    lt = a2pool.tile([128, NW], f32, tag="lm_out")
                    nc.vector.tensor_copy(lt[:], ps[:])
                    nc.sync.dma_start(
                        d_logits.ap()[TRUE[s] * 128:(TRUE[s] + 1) * 128,
                                      ni * NW:(ni + 1) * NW], lt[:])
                    nc.scalar.activation(scr[:], ps[:], AF.Exp,
                                         accum_out=sums[s][:, ni:ni + 1])
            for s in range(NCH):
                es = apool.tile([128, 1], f32, tag="es")
                nc.vector.tensor_reduce(es[:], sums[s][:],
                                        axis=mybir.AxisListType.X, op=ALU.add)
                nc.sync.dma_start(
                    d_expsum.ap()[TRUE[s] * 128:(TRUE[s] + 1) * 128, :], es[:])

    nc.compile()
    return nc


def _fold_col_tiles(w, nb):
    """[C, nb*128] -> [nb, 128, C] with out[b, p, c*128+f] = w[c*128+p, b*128+f]"""
    cc = w.shape[0] // 128
    return np.ascontiguousarray(
        w.reshape(cc, 128, nb, 128).transpose(2, 1, 0, 3).reshape(nb, 128, cc * 128))


def _prep(inputs, nlayer):
    """Host-side weight folding/repacking. Returns per-core in_maps."""
    g = {}
    for k, v in inputs.items():
        a = np.asarray(v)
        g[k] = a if a.dtype in (np.int64, np.int32) else a.astype(np.float32)
    idx, targets = g["idx"], g["targets"]
    x0 = g["wte"][idx] + g["wpe"][:T][None, :, :]           # [B, T, C] f32

    wq_t = np.empty((nlayer, NCC, 128, C), np.float32)
    wk_t = np.empty((nlayer, NCC, 128, C), np.float32)
    wv_t = np.empty((nlayer, NCC, 128, C), np.float32)
    apw_t = np.empty((nlayer, NCC, 128, C), np.float32)
    fcw_t = np.empty((nlayer, NFB, 128, C), np.float32)
    prw_t = np.empty((nlayer, NFB, 128, C), np.float32)
    bqk = np.zeros((nlayer, 128, 12), np.float32)
    fcb = np.zeros((nlayer, 128, NFB), np.float32)
    brows = np.zeros((nlayer, 3, C), np.float32)
    for l in range(nlayer):
        wq = g["ln1_g"][l][:, None] * g["wq"][l]
        wk = g["ln1_g"][l][:, None] * g["wk"][l]
        wv = g["ln1_g"][l][:, None] * g["wv"][l]
        fw = g["ln2_g"][l][:, None] * g["fc_w"][l]
        fb = g["fc_b"][l] + g["ln2_b"][l] @ g["fc_w"][l]
        wq_t[l] = _fold_col_tiles(wq, NCC)
        wk_t[l] = _fold_col_tiles(wk, NCC)
        wv_t[l] = wv.reshape(NCC, 128, C)
        apw_t[l] = g["attn_pw"][l].reshape(NCC, 128, C)
        fcw_t[l] = _fold_col_tiles(fw, NFB)
        prw_t[l] = g["pr_w"][l].reshape(NFB, 128, C)
        bqk[l, :, 0:6] = (g["ln1_b"][l] @ g["wq"][l]).reshape(6, 128).T
        bqk[l, :, 6:12] = (g["ln1_b"][l] @ g["wk"][l]).reshape(6, 128).T
        fcb[l] = fb.reshape(NFB, 128).T
        brows[l, 0] = g["ln1_b"][l] @ g["wv"][l]
        brows[l, 1] = g["attn_pb"][l]
        brows[l, 2] = g["pr_b"][l]

    lmw = g["lnf_g"][:, None] * g["lm_w"]                   # [C, V]
    lmb_full = g["lnf_b"] @ g["lm_w"]                       # [V]
    lmw_pad = np.zeros((C, VPAD), np.float32)
    lmw_pad[:, :V] = lmw
    lmb_pad = np.zeros((VPAD,), np.float32)
    lmb_pad[:V] = lmb_full

    use_bias = bool(np.any(brows != 0) or np.any(lmb_pad != 0))

    shared = dict(wq=wq_t, wk=wk_t, wv=wv_t, apw=apw_t, fcw=fcw_t, prw=prw_t,
                  bqk=bqk, fcb=fcb, brows=brows,
                  cident=np.eye(128, dtype=np.float32),
                  cones=np.ones((128, 128), np.float32),
                  conesb=np.ones((128, 24), ml_dtypes.bfloat16))

    per_core = []
    for core in range(8):
        grp, r = core // 4, core % 4
        oc = [r, 7 - r]
        x0_own = np.concatenate(
            [x0[grp, c * 128:(c + 1) * 128] for c in oc], axis=0)
        mask = np.zeros((NCH, 128, 256), np.float32)
        for s in range(NCH):
            for qi, cq in enumerate(oc):
                ck = TRUE[s]
                if ck < cq:
                    mask[s, :, qi * 128:(qi + 1) * 128] = 1.0
                elif ck == cq:
                    tri = (np.arange(128)[None, :] >= np.arange(128)[:, None])
                    mask[s, :, qi * 128:(qi + 1) * 128] = tri.astype(np.float32)
        sh = lmw_pad[:, r * VSH:(r + 1) * VSH]
        lmw_core = np.ascontiguousarray(
            sh.reshape(NCC, 128, NN, NW).transpose(2, 1, 0, 3)
            .reshape(NN, 128, NCC * NW))
        per_core.append(dict(
            x0=np.ascontiguousarray(x0_own.reshape(2, 128, C)),
            mask=mask.astype(ml_dtypes.bfloat16), lmw=lmw_core,
            lmb=lmb_pad[r * VSH:(r + 1) * VSH].reshape(1, VSH), **shared))
    return per_core, idx, targets, use_bias


def assemble(results, idx, targets):
    logits = np.empty((B, T, V), np.float32)
    expsum = np.zeros((B, T), np.float64)
    npad_last = VPAD - V                                    # pads in rank-3 shard
    for core in range(8):
        grp, r = core // 4, core % 4
        sh = results[core]["logits"]                        # [T, VSH]
        lo = r * VSH
        hi = min((r + 1) * VSH, V)
        if lo < V:
            logits[grp, :, lo:hi] = sh[:, :hi - lo]
        es = results[core]["expsum"][:, 0].astype(np.float64)
        if r == 3:
            es = es - npad_last                             # pad cols give exp(0)=1
        expsum[grp] += es
    lse = np.log(expsum)                                    # [B, T]
    tl = np.take_along_axis(logits, np.asarray(targets)[..., None], axis=-1)[..., 0]
    loss = np.float32(np.mean(lse - tl.astype(np.float64)))
    return logits, loss


def run(inputs, trace=False):
    nlayer = _NLAYER
    per_core, idx, targets, use_bias = _prep(inputs, nlayer)
    key = (nlayer, use_bias)
    if key not in _cache:
        _cache[key] = _build(nlayer, use_bias)
    nc = _cache[key]
    res = run_bass_kernel_spmd(nc, per_core, core_ids=list(range(8)),
                               trace=trace)
    out = assemble(res.results, idx, targets)
    return out, res


def kernel(**inputs):
    out, _ = run(inputs, trace=False)
    return out
